# revision 27
# baseline (speedup 1.0000x reference)
"""Trainium2 Bass kernel for CapsuleLayer (dynamic routing) on 8 NeuronCores.

Problem: x[32,1152,64], W[1152,32,64,64], bias[1,1152,32,1] (zeros) ->
         out[32,32,64]
  inputs_hat = einsum('bip,icpq->bicq', x, W)
  3 rounds of routing (softmax over capsule axis, squash, agreement update).

Sharding: input-capsule axis i=1152 split over 8 cores (144 each).

Phase 1 per core (72 pairs, 2 i's per K=128 matmul): consecutive pairs share
one [128, 2048] PSUM tile -- the even pair's stationary occupies columns
0:64 (out rows 0:64), the odd pair's columns 64:128 (out rows 64:128, PSUM
accumulate over a zeroed stationary half), so the PSUM->SBUF bf16 cast runs
at full 128-partition width and no extra matmuls are needed for the round-0
sum: s0 = sum_i ih is accumulated from the bf16 ih tiles on the otherwise
idle GpSimd engine (fp32, split in two chunks so the first AllReduce hides
under phase 1), then folded 128->32 partitions with a 1/32-selector matmul.
Most ih tiles stay resident in SBUF; the first few sweeps' tiles spill to
HBM and are prefetched back during the round transition.

Routing rounds r=1,2 (36 sweeps of 4 i's x 32 b = 128 partitions):
  DVE:   pr = ih*v (bf16 2x), pairwise-add tree level 1 + grouped reduce
         over the last 16 q -> dlog (the bf16 2x tree halves the volume
         seen by the non-2x TensorReduce)
  GpSimd: tree level 2, wtt = ih * exp(logits) with the exp read broadcast
         stride-0 (no materialized [128,2048] e4), logit accumulate,
         selector scale
  ACT:   exp with Z accumulator
  PE:    4 selector matmuls accumulate s into PSUM
Per-round partial sums are AllReduced in bf16, split 30/6 sweeps so the
first collective hides under compute. The final round ReduceScatters in
fp32 instead; each core squashes and writes only its 4-batch output shard,
which the host concatenates.
"""

import os
import sys

import numpy as np

for _p in (
    "/opt/trn_rl_repo",
    "/root/.axon_site",
    "/root/.axon_site/_ro/trn_rl_repo",
    "/root/.axon_site/_ro/pypackages",
):
    if os.path.isdir(_p) and _p not in sys.path:
        sys.path.append(_p)

import ml_dtypes
import concourse.bacc as bacc
import concourse.mybir as mybir
import concourse.tile as tile
from concourse.bass_utils import run_bass_kernel_spmd

F32 = mybir.dt.float32
BF16 = mybir.dt.bfloat16
AF = mybir.ActivationFunctionType
AX = mybir.AxisListType
ALU = mybir.AluOpType
BF = ml_dtypes.bfloat16

B, I, P, C, Q = 32, 1152, 64, 32, 64
N_CORES = 8
IL = I // N_CORES          # 144 input capsules per core
NPAIR = IL // 2            # 72 matmul pairs
NSWEEP = IL // 4           # 36 routing sweeps (4 i's x 32 b = 128 partitions)
NTILE = NSWEEP             # one [128, 2048] ih tile per 2 pairs
CQ = C * Q                 # 2048
BSH = B // N_CORES         # 4 output batches per core (ReduceScatter shard)
NUM_ROUTING = 3

CONFIG = {
    "trace": False,           # profile the run (exec_time_ns); needs ntff hook
    "trace_cores": None,      # None -> core 0 only
    "nspill": 36,              # ih tiles spilled to HBM (rest stay in SBUF)
    "split_tile": 24,         # s0 partial AllReduced after this many ih tiles
    "split_sweep": 30,        # routing partial AllReduced after this sweep
}

_compiled = None
_compiled_cfg = None


def _build_kernel():
    """Build + compile the SPMD Bass module (identical program on 8 cores)."""
    nc = bacc.Bacc("TRN2", target_bir_lowering=False, debug=False,
                   num_devices=N_CORES)

    NSPILL = CONFIG["nspill"]
    SPLIT_TILE = CONFIG["split_tile"]
    SPLIT_SWEEP = CONFIG["split_sweep"]

    # lhsT_all[p, t*128+m]: stationary for pair t; even t uses cols 0:64,
    # odd t cols 64:128, the other half zeros (out-row routing via columns).
    lall_d = nc.dram_tensor("lhsT", [128, NPAIR * 128], BF16,
                            kind="ExternalInput")
    w_d = nc.dram_tensor("w_rhs", [NPAIR, 128, CQ], BF16, kind="ExternalInput")
    # sel[:, 0:32] = tiled eye(32); sel[:, 32:64] = tiled eye(32)/32
    sel_d = nc.dram_tensor("sel", [128, 64], F32, kind="ExternalInput")
    out_d = nc.dram_tensor("out", [BSH, CQ], F32, kind="ExternalOutput")
    ih_d = nc.dram_tensor("ih_spill", [max(NSPILL, 1), 128, CQ], BF16)

    rgroups = [list(range(N_CORES))]

    with tile.TileContext(nc) as tc:
        with (
            tc.tile_pool(name="ihsb", bufs=1) as ihsb_pool,
            tc.tile_pool(name="small", bufs=3) as small_pool,
            tc.tile_pool(name="bacc", bufs=1) as bacc_pool,
            tc.tile_pool(name="sv", bufs=2) as sv_pool,
            tc.tile_pool(name="ps", bufs=8, space="PSUM") as ps_pool,
            tc.tile_pool(name="dram", bufs=2, space="DRAM") as dram_pool,
        ):
            sel_t = small_pool.tile([128, 64], F32, tag="sel")
            nc.sync.dma_start(sel_t[:], sel_d[:])
            sel_bf = small_pool.tile([128, 32], BF16, tag="selbf")
            nc.vector.tensor_copy(sel_bf[:], sel_t[:, 0:32])
            sel32 = small_pool.tile([128, 32], BF16, tag="sel32")
            nc.vector.tensor_copy(sel32[:], sel_t[:, 32:64])  # eye/32, bf16

            b_acc = bacc_pool.tile([128, NSWEEP * 32], F32, tag="bacc")
            nc.vector.memset(b_acc[:], 0.0)

            # Warm-up collectives: the first one pays a large one-time
            # rendezvous cost; burn both flavors here, hidden under phase 1.
            wu_sb = small_pool.tile([32, 16], F32, tag="wu")
            nc.vector.memset(wu_sb[:], 0.0)
            wu_in = dram_pool.tile([32, 16], F32, tag="wu_in", bufs=1)
            wu_out = dram_pool.tile([32, 16], F32, tag="wu_out", bufs=1)
            wu_rs = dram_pool.tile([4, 16], F32, tag="wu_rs", bufs=1)
            nc.gpsimd.dma_start(wu_in[:], wu_sb[:])
            nc.gpsimd.collective_compute(
                "AllReduce", ALU.add,
                ins=[wu_in[:].opt()], outs=[wu_out[:].opt()],
                replica_groups=rgroups,
            )
            nc.gpsimd.collective_compute(
                "ReduceScatter", ALU.add,
                ins=[wu_out[:].opt()], outs=[wu_rs[:].opt()],
                replica_groups=rgroups,
            )

            # ih tiles: tile k covers pairs (2k, 2k+1) = 4 i's, [128, CQ]
            # partition = (i_slot*32 + b), free = (c, q).
            ih_tiles = [
                None if k < NSPILL else
                ihsb_pool.tile([128, CQ], BF16, tag=f"ih{k}",
                               name=f"ih_sb_{k}")
                for k in range(NTILE)
            ]

            def flush_ar(blks, tag, final):
                """4 PSUM blocks [32,512] -> SBUF bf16 -> DRAM ->
                AllReduce, or ReduceScatter to a 4-batch shard (final)."""
                f_sb = sv_pool.tile([32, CQ], BF16, name=f"fsb_{tag}",
                                    tag="f16", bufs=2)
                for j in range(4):
                    dst = f_sb[:, 512 * j:512 * (j + 1)]
                    if j % 2 == 0:
                        nc.scalar.copy(dst, blks[j][0:32, :])
                    else:
                        nc.vector.tensor_copy(dst, blks[j][0:32, :])
                a_in = dram_pool.tile([32, CQ], BF16, tag=tag + "_in", bufs=1)
                nc.gpsimd.dma_start(a_in[:], f_sb[:])
                rtag = "redA" if tag.endswith("a") else "redB"
                if final:
                    a_out = dram_pool.tile([BSH, CQ], BF16, tag=tag + "_out",
                                           bufs=1)
                    nc.gpsimd.collective_compute(
                        "ReduceScatter", ALU.add,
                        ins=[a_in[:].opt()], outs=[a_out[:].opt()],
                        replica_groups=rgroups,
                    )
                    red = sv_pool.tile([BSH, 4, C, 16], BF16,
                                       tag=rtag, bufs=2, name=f"red_{tag}")
                else:
                    a_out = dram_pool.tile([32, CQ], BF16, tag=tag + "_out",
                                           bufs=1, addr_space="Shared")
                    nc.gpsimd.collective_compute(
                        "AllReduce", ALU.add,
                        ins=[a_in[:].opt()], outs=[a_out[:].opt()],
                        replica_groups=rgroups,
                    )
                    red = sv_pool.tile([32, 4, C, 16], BF16,
                                       tag=rtag, bufs=2, name=f"red_{tag}")
                nc.gpsimd.dma_start(red[:], a_out[:])
                return red

            # ---------------- Phase 1: ih = x @ W; s0 accumulation ----------
            with (
                tc.tile_pool(name="w", bufs=6) as w_pool,
                tc.tile_pool(name="lt", bufs=1) as lt_pool,
                tc.tile_pool(name="acc", bufs=1) as acc_pool,
            ):
                lall = lt_pool.tile([128, NPAIR * 128], BF16, tag="lall")
                nc.sync.dma_start(lall[:], lall_d[:])
                accA = acc_pool.tile([128, CQ], F32, tag="accA")
                accB = acc_pool.tile([128, CQ], F32, tag="accB")
                nc.gpsimd.memset(accA[:], 0.0)
                nc.gpsimd.memset(accB[:], 0.0)

                def fold_s0(acc, tag):
                    """128->32 partition fold of acc via 1/32-selector
                    matmuls (bf16 to keep the PE bubble short) into borrowed
                    PSUM slots, then flush+AllReduce."""
                    accbf = acc_pool.tile([128, CQ], BF16, tag="accbf",
                                          bufs=1, name=f"accbf_{tag}")
                    nc.scalar.copy(accbf[:], acc[:])
                    blks = []
                    for j in range(4):
                        pb = ps_pool.tile([128, 512], F32, tag="mm",
                                          name=f"fold_{tag}_{j}")
                        nc.tensor.matmul(pb[0:32, :], sel32[:],
                                         accbf[:, 512 * j:512 * (j + 1)],
                                         start=True, stop=True)
                        blks.append(pb)
                    return flush_ar(blks, tag, final=False)

                ar_handles = []
                for k in range(NTILE):
                    blks = [ps_pool.tile([128, 512], F32, tag="mm",
                                         name=f"mm_{k}_{j}")
                            for j in range(4)]
                    for half in range(2):      # even / odd pair of the tile
                        t = 2 * k + half
                        wt_ = w_pool.tile([128, CQ], BF16, tag="w",
                                          name=f"w_{t}")
                        nc.sync.dma_start(wt_[0:64, :], w_d[t, 0:64])
                        nc.sync.dma_start(wt_[64:128, :], w_d[t, 64:128])
                        lt = lall[:, t * 128:(t + 1) * 128]
                        for j in range(4):
                            nc.tensor.matmul(
                                blks[j][:, :], lt[:],
                                wt_[:, 512 * j:512 * (j + 1)],
                                start=(half == 0), stop=(half == 1),
                            )
                    if ih_tiles[k] is None:
                        it = sv_pool.tile([128, CQ], BF16, tag="spill_c",
                                          bufs=4, name=f"ih_spill_sb_{k}")
                    else:
                        it = ih_tiles[k]
                    for j in range(4):
                        dst = it[:, 512 * j:512 * (j + 1)]
                        if j < 2:
                            nc.scalar.copy(dst, blks[j][:, :])
                        else:
                            nc.vector.tensor_copy(dst, blks[j][:, :])
                    if ih_tiles[k] is None:
                        nc.scalar.dma_start(ih_d[k], it[:])
                    acc = accA if k < SPLIT_TILE else accB
                    # all on V: anything queued behind the warm-up
                    # collectives on the gpsimd queue stalls the pipeline
                    # for the ~80us first-collective rendezvous
                    nc.vector.tensor_add(acc[:], acc[:], it[:])
                    if k == SPLIT_TILE - 1:
                        ar_handles.append(fold_s0(accA, "s0a"))
                ar_handles.append(fold_s0(accB, "s0b"))

            # ---------------- Routing rounds -------------------------------
            with (
                tc.tile_pool(name="itp", bufs=6) as it_pool,
                tc.tile_pool(name="prod", bufs=2) as prod_pool,
                tc.tile_pool(name="wt", bufs=2) as wt_pool,
                tc.tile_pool(name="tree", bufs=2) as tree_pool,
                tc.tile_pool(name="v4", bufs=1) as v4_pool,
                tc.tile_pool(name="sq", bufs=1) as sq_pool,
            ):
                for r in range(1, NUM_ROUTING + 1):
                    final = (r == NUM_ROUTING)
                    pa, pb = ar_handles
                    npart = BSH if final else 32
                    sdt = BF16
                    # S and squash on b-partitions, free = (qh=4, c, ql=16)
                    S_sb = sq_pool.tile([32, 4, C, 16], sdt, tag="S_bf",
                                        name="S_sb")[0:npart]
                    nc.vector.tensor_add(S_sb[:], pa[:], pb[:])
                    sqr = sq_pool.tile([32, 4, C, 16], sdt, tag="sqr_bf",
                                       name="sqr")[0:npart]
                    nc.vector.tensor_mul(sqr[:], S_sb[:], S_sb[:])
                    q1 = sq_pool.tile([32, 2, C, 16], sdt, tag="q1",
                                      name="q1")[0:npart]
                    nc.vector.tensor_add(q1[:], sqr[:, 0:2], sqr[:, 2:4])
                    q2 = sq_pool.tile([32, C, 16], sdt, tag="q2",
                                      name="q2")[0:npart]
                    nc.vector.tensor_add(q2[:], q1[:, 0], q1[:, 1])
                    sq = small_pool.tile([32, C], F32, tag="sq",
                                         name="sq")[0:npart]
                    nc.vector.reduce_sum(sq[:], q2[:], axis=AX.X)
                    lnq = small_pool.tile([32, C], F32, tag="lnq",
                                          name="lnq")[0:npart]
                    nc.scalar.activation(lnq[:], sq[:], AF.Ln)
                    rt = small_pool.tile([32, C], F32, tag="rt",
                                         name="rt")[0:npart]
                    nc.scalar.activation(rt[:], lnq[:], AF.Exp, scale=0.5)
                    onep = small_pool.tile([32, C], F32, tag="onep",
                                           name="onep")[0:npart]
                    nc.vector.tensor_scalar_add(onep[:], sq[:], 1.0)
                    rden = small_pool.tile([32, C], F32, tag="rden",
                                           name="rden")[0:npart]
                    nc.vector.reciprocal(rden[:], onep[:])
                    scale = small_pool.tile([32, C], F32, tag="scale",
                                            name="scale")[0:npart]
                    nc.vector.tensor_mul(scale[:], rt[:], rden[:])
                    # materialize the broadcast scale on ACT (stride-0 DVE
                    # reads are slow on hw)
                    sc64 = sq_pool.tile([32, 4, C, 16], sdt, tag="sc64",
                                        name="sc64")[0:npart]
                    nc.scalar.activation(
                        sc64[:],
                        scale[:].unsqueeze(1).unsqueeze(-1).broadcast_to(
                            (npart, 4, C, 16)), AF.Copy)

                    if final:
                        out_sb = sv_pool.tile([BSH, 4, C, 16], F32,
                                              tag="out_sb", bufs=1)
                        nc.vector.tensor_mul(out_sb[:], S_sb[:], sc64[:])
                        nc.sync.dma_start(
                            out_d[:],
                            out_sb[:].rearrange("b a c l -> b (a c l)"))
                        break

                    v_c = sv_pool.tile([32, CQ], BF16, tag="v_c", bufs=1)
                    nc.vector.tensor_mul(
                        v_c[:].rearrange("b (a c l) -> b a c l", c=C, l=16),
                        S_sb[:], sc64[:])
                    v4 = v4_pool.tile([128, CQ], BF16, tag="v4")
                    engs = [nc.gpsimd, nc.scalar, nc.sync, nc.gpsimd]
                    for g in range(4):
                        engs[g].dma_start(v4[32 * g:32 * (g + 1), :], v_c[:])
                    v43 = v4[:].rearrange("p (a c l) -> p a c l", c=C, l=16)

                    use_rs = (r == NUM_ROUTING - 1)
                    ar_handles = []
                    blksA = [ps_pool.tile([128, 512], F32, tag="mm",
                                          name=f"s_{r}_0_{j}")
                             for j in range(4)]
                    blksB = None
                    pend = {}

                    def apply_stage(si):
                        """selector scale + coefficient multiply + matmuls
                        for sweep si (emitted one sweep late so the Pool/PE
                        never stall on the ACT exp round trip)."""
                        nonlocal blksB
                        it3_, e_, z_ = pend.pop(si)
                        blks = blksA if si < SPLIT_SWEEP else blksB
                        first = si == 0 or si == SPLIT_SWEEP
                        last_s = si == SPLIT_SWEEP - 1 or si == NSWEEP - 1
                        rz = small_pool.tile([128, 1], F32, tag="rz")
                        nc.vector.reciprocal(rz[:], z_[:])
                        selz = small_pool.tile([128, 32], BF16, tag="selz")
                        nc.scalar.activation(selz[:], sel_bf[:], AF.Copy,
                                             scale=rz[:])
                        # e broadcast over q_lo only: each 512-col PSUM
                        # block is one q_hi slice, so all share this pattern
                        e5 = wt_pool.tile([128, C, 16], BF16, tag="e5")
                        nc.scalar.activation(
                            e5[:],
                            e_[:].unsqueeze(-1).broadcast_to((128, C, 16)),
                            AF.Copy)
                        wtt = wt_pool.tile([128, 4, C, 16], BF16,
                                           tag="wtt")
                        nc.vector.tensor_mul(
                            wtt[:], it3_,
                            e5[:].unsqueeze(1).broadcast_to((128, 4, C, 16)))
                        wt2 = wtt[:].rearrange("p a c l -> p (a c l)")
                        for j in range(4):
                            nc.tensor.matmul(
                                blks[j][0:32, :], selz[:],
                                wt2[:, 512 * j:512 * (j + 1)],
                                start=first, stop=last_s)
                        if last_s:
                            ar_handles.append(flush_ar(
                                blks, f"r{r}" + ("a" if si < NSWEEP - 1
                                                 else "b"), final=use_rs))

                    for s in range(NSWEEP):
                        if s == SPLIT_SWEEP:
                            blksB = [ps_pool.tile([128, 512], F32, tag="mm",
                                                  name=f"s_{r}_1_{j}")
                                     for j in range(4)]
                        if ih_tiles[s] is None:
                            it = it_pool.tile([128, CQ], BF16, tag="it",
                                              name=f"it_{r}_{s}")
                            nc.sync.dma_start(it[0:64, :], ih_d[s, 0:64])
                            nc.sync.dma_start(it[64:128, :],
                                              ih_d[s, 64:128])
                        else:
                            it = ih_tiles[s]
                        it3 = it[:].rearrange("p (a c l) -> p a c l",
                                              c=C, l=16)

                        # logit update: dlog = sum_q it*v  (contiguous bf16
                        # 2x add tree over q_hi, then grouped reduce of q_lo)
                        pr = prod_pool.tile([128, 4, C, 16], BF16, tag="pr")
                        nc.vector.tensor_mul(pr[:], it3, v43)
                        p1 = tree_pool.tile([128, 2, C, 16], BF16, tag="p1")
                        nc.vector.tensor_add(p1[:], pr[:, 0:2], pr[:, 2:4])
                        p2 = tree_pool.tile([128, C, 16], BF16, tag="p2")
                        nc.vector.tensor_add(p2[:], p1[:, 0], p1[:, 1])
                        dlog = small_pool.tile([128, C], F32, tag="dlog")
                        nc.vector.reduce_sum(dlog[:], p2[:], axis=AX.X)
                        bsl = b_acc[:, 32 * s:32 * (s + 1)]
                        nc.vector.tensor_add(bsl, bsl, dlog[:])

                        # softmax over c: coef = exp(b)/Z; 1/Z folded into
                        # the selector weights, exp read broadcast by gpsimd.
                        e = small_pool.tile([128, C], BF16, tag="e")
                        z = small_pool.tile([128, 1], F32, tag="z")
                        nc.scalar.activation(e[:], bsl, AF.Exp,
                                             accum_out=z[:])
                        pend[s] = (it3, e, z)
                        if s > 0:
                            apply_stage(s - 1)
                    apply_stage(NSWEEP - 1)

    nc.compile()
    return nc


def _prep_core_inputs(x, W):
    """Host-side shard + repack for one call. Returns list of in_maps."""
    xs_all = np.ascontiguousarray(x)          # [B, I, P]
    in_maps = []
    eye = np.eye(32, dtype=np.float32)
    sel = np.concatenate([np.tile(eye, (4, 1)),
                          np.tile(eye / C, (4, 1))], axis=1)  # [128, 64]
    for k in range(N_CORES):
        xs = xs_all[:, k * IL:(k + 1) * IL, :]          # [B, IL, P]
        xt = xs.transpose(1, 2, 0).reshape(NPAIR, 2, P, B)  # [t, i2, p, b]
        lhsT = np.zeros((NPAIR, 128, 128), np.float32)
        # stationary col m = 64*(t%2) + 32*i2 + b  ->  out row m
        for i2 in range(2):
            rows = slice(64 * i2, 64 * (i2 + 1))        # K rows (i2, p)
            for par in range(2):                        # even/odd pairs
                cols = slice(64 * par + 32 * i2, 64 * par + 32 * (i2 + 1))
                lhsT[par::2, rows, cols] = xt[par::2, i2]
        lall = np.ascontiguousarray(
            lhsT.astype(BF).transpose(1, 0, 2)).reshape(128, -1)
        Ws = W[k * IL:(k + 1) * IL]                      # [IL, C, P, Q]
        # free-axis layout (q_hi=4, c=32, q_lo=16): contiguous add-tree
        w_rhs = np.ascontiguousarray(
            Ws.reshape(NPAIR, 2, C, P, 4, 16).transpose(0, 1, 3, 4, 2, 5)
            .reshape(NPAIR, 128, CQ).astype(BF))
        in_maps.append({"lhsT": lall, "w_rhs": w_rhs, "sel": sel})
    return in_maps


def _host_reference(x, W, bias):
    """Exact numpy fallback (used only if bias != 0, which the problem's
    input spec says cannot happen; the device kernel assumes uniform
    round-0 routing coefficients)."""
    ih = np.einsum("bip,icpq->bicq", x, W)
    b = bias.astype(np.float64)
    out = None
    for r in range(NUM_ROUTING):
        e = np.exp(b - b.max(axis=2, keepdims=True))
        c = e / e.sum(axis=2, keepdims=True)
        s = (c * ih).sum(axis=1, keepdims=True)
        sq = np.sum(s * s, axis=-1, keepdims=True)
        out = s * (sq / (1.0 + sq) / np.sqrt(sq))
        if r != NUM_ROUTING - 1:
            b = b + np.sum(ih * out, axis=-1, keepdims=True)
    return out.reshape(B, C, Q).astype(np.float32)


def kernel(x, W, bias):
    global _compiled, _compiled_cfg
    x = np.asarray(x, dtype=np.float32)
    W = np.asarray(W, dtype=np.float32)
    bias = np.asarray(bias, dtype=np.float32)
    if np.any(bias):
        return _host_reference(x, W, bias)

    cfg = (CONFIG["nspill"], CONFIG["split_tile"], CONFIG["split_sweep"])
    if _compiled is None or _compiled_cfg != cfg:
        _compiled = _build_kernel()
        _compiled_cfg = cfg
    nc = _compiled

    in_maps = _prep_core_inputs(x, W)
    # untraced warm-up execution: the first NEFF run in a session lands on
    # a cold device clock and measures ~10% slow
    run_bass_kernel_spmd(nc, in_maps, list(range(N_CORES)))
    res = run_bass_kernel_spmd(
        nc, in_maps, list(range(N_CORES)),
        trace=CONFIG["trace"], trace_cores=CONFIG["trace_cores"],
    )
    kernel.last_results = res
    out = np.concatenate([res.results[k]["out"] for k in range(N_CORES)],
                         axis=0)
    # device layout is (q_hi=4, c=32, q_lo=16) along the free axis
    return np.ascontiguousarray(
        out.reshape(B, 4, C, 16).transpose(0, 2, 1, 3).reshape(B, C, Q))
